# revision 13
# baseline (speedup 1.0000x reference)
"""ErrorAwareEdgeLoss Trainium2 kernel (split-engine version).

Math: loss = mean_b [ (sum_e w_be * P[b,i_e,:] @ D @ P[b,j_e,:]) / max(sum_e w_be, 1e-8) ]

Reformulation:
    G_b = (P_b @ D) @ P_b^T          (bf16 matmuls on the PE)
    sum_e w_e * G_b[i_e, j_e] splits across two engines:

  * Gather path (first EG edges): G_b spills to DRAM as a [512, 128]-bf16
    token table; SWDGE dma_gather fetches one 256B token per edge; a
    host-prebuilt mask M (w_e at the edge's lane, 0 elsewhere) turns the
    lane-select + weight into one fused DVE multiply-reduce.
  * Scatter-matmul path (remaining EW edges): numerator partial is
    <W_b, G_b> where W_b[n,m] = sum_e w_e 1{i_e=n} 1{j_e=m}. W_b is built
    ON the PE as OneHot_i^T @ (w*OneHot_j) from host-prebuilt fp8 one-hot
    operands (exact 0/1 + fp8-rounded w), then dotted with G_b on the DVE.

The two paths run on different engines (Pool vs PE) and overlap across
batches. Input loads stream on the sync queue with no producer deps; the
G spill rides the scalar queue so it never blocks input prefetch.

Sharding: data-parallel over batch: 8 NeuronCores x 8 batches. Each core
emits a partial sum of per-sample losses; the host adds the 8 partials and
divides by B (the all-reduce of the sharding hint).
"""

from contextlib import ExitStack

import ml_dtypes
import numpy as np

import concourse.bacc as bacc
import concourse.bass as bass
import concourse.mybir as mybir
import concourse.tile as tile
from concourse.bass_utils import run_bass_kernel_spmd

B, N, E = 64, 256, 8192
NCORES = 8
BPC = B // NCORES  # batches per core

EG = 2048  # edges via the gather path (per batch)
EW = E - EG  # edges via the scatter-matmul path
KC = EW // 128  # contraction chunks for the W build
TOKB = 64  # f32 elems per gathered token (256B rows)
NTOK = N * N // TOKB  # 1024
QG = EG // 128  # gather output slots per partition

f32 = mybir.dt.float32
bf16 = mybir.dt.bfloat16
fp8 = mybir.dt.float8e4
i16 = mybir.dt.int16

NP_BF16 = ml_dtypes.bfloat16
NP_FP8 = ml_dtypes.float8_e4m3


def _build_bass():
    nc = bacc.Bacc("TRN2", target_bir_lowering=False, debug=False,
                   num_swdge_queues=4, dynamic_dma_scratch_size=98304)

    pt_in = nc.dram_tensor("pt", [BPC, 128, 2, N], bf16, kind="ExternalInput")
    d_in = nc.dram_tensor("derr", [128, 2, N], bf16, kind="ExternalInput")
    ti_in = nc.dram_tensor("ti", [BPC, 128, EG // 16], i16, kind="ExternalInput")
    m_in = nc.dram_tensor("mk", [BPC, 128, QG, TOKB], fp8, kind="ExternalInput")
    wi_in = nc.dram_tensor("wi", [BPC, 128, KC, N], fp8, kind="ExternalInput")
    ww_in = nc.dram_tensor("ww", [BPC, 128, KC, N], fp8, kind="ExternalInput")
    ew_in = nc.dram_tensor("ew", [BPC, 128, E // 128], f32, kind="ExternalInput")
    out = nc.dram_tensor("out", [1, 1], f32, kind="ExternalOutput")

    with tile.TileContext(nc) as tc, ExitStack() as ctx:
        const_pool = ctx.enter_context(tc.tile_pool(name="const", bufs=1))
        pt_pool = ctx.enter_context(tc.tile_pool(name="pt", bufs=BPC))
        sm_pool = ctx.enter_context(tc.tile_pool(name="sm", bufs=BPC))
        qt_pool = ctx.enter_context(tc.tile_pool(name="qt", bufs=2))
        g_pool = ctx.enter_context(tc.tile_pool(name="g", bufs=2))
        w_pool = ctx.enter_context(tc.tile_pool(name="w", bufs=2))
        oh_pool = ctx.enter_context(tc.tile_pool(name="oh", bufs=3))
        e_pool = ctx.enter_context(tc.tile_pool(name="edges", bufs=3))
        tok_pool = ctx.enter_context(tc.tile_pool(name="tok", bufs=2))
        psum_pool = ctx.enter_context(tc.tile_pool(name="ps", bufs=2, space="PSUM"))
        dram_pool = ctx.enter_context(tc.tile_pool(name="dram", bufs=3, space="DRAM"))

        d_sb = const_pool.tile([128, 2, N], bf16)
        nc.sync.dma_start(d_sb[:], d_in[:])
        ones_sb = const_pool.tile([128, 1], f32)
        nc.vector.memset(ones_sb[:], 1.0)
        # per-batch partials: [0,BPC) gather numer, [BPC,2B) W numer, [2B,3B) wsum
        red_sb = const_pool.tile([128, 3 * BPC], f32)

        # ---- preload every batch's small inputs up front (sync queue head):
        # the PE/gather pipelines never wait behind the bulk mask streams.
        pt_all, ti_all, ew_all = [], [], []
        for b in range(BPC):
            pt_sb = pt_pool.tile([128, 2, N], bf16, tag="pt")
            nc.sync.dma_start(pt_sb[:], pt_in[b])
            ti_sb = sm_pool.tile([128, EG // 16], i16, tag="ti")
            nc.sync.dma_start(ti_sb[:], ti_in[b])
            ew_sb = sm_pool.tile([128, E // 128], f32, tag="ew")
            nc.sync.dma_start(ew_sb[:], ew_in[b])
            pt_all.append(pt_sb)
            ti_all.append(ti_sb)
            ew_all.append(ew_sb)

        for b in range(BPC):
            pt_sb, ti_sb, ew_sb = pt_all[b], ti_all[b], ew_all[b]
            # bulk streams on sync: mask first (gather path is latency-critical)
            m_sb = e_pool.tile([128, QG, TOKB], fp8, tag="mk")
            nc.sync.dma_start(m_sb[:], m_in[b])
            wi_sb = oh_pool.tile([128, KC, N], fp8, tag="wi")
            ww_sb = oh_pool.tile([128, KC, N], fp8, tag="ww")
            nc.sync.dma_start(wi_sb[:], wi_in[b])
            nc.sync.dma_start(ww_sb[:], ww_in[b])

            # ---- QT[n, i] = Q[i, n], Q = P @ D
            qt_sb = qt_pool.tile([128, 2, N], bf16)
            for ncx in range(2):
                qt_ps = psum_pool.tile([128, N], f32, tag="qtps")
                for kc in range(2):
                    nc.tensor.matmul(
                        qt_ps[:],
                        lhsT=d_sb[:, kc, ncx * 128 : (ncx + 1) * 128],
                        rhs=pt_sb[:, kc, :],
                        start=(kc == 0),
                        stop=(kc == 1),
                    )
                nc.scalar.copy(qt_sb[:, ncx, :], qt_ps[:])

            # ---- G[i, j] = sum_n QT[n, i] PT[n, j]; g_sb[p, ic, j] = G[ic*128+p, j]
            g_sb = g_pool.tile([128, 2, N], f32)
            for ic in range(2):
                g_ps = psum_pool.tile([128, N], f32, tag="gps")
                for ncx in range(2):
                    nc.tensor.matmul(
                        g_ps[:],
                        lhsT=qt_sb[:, ncx, ic * 128 : (ic + 1) * 128],
                        rhs=pt_sb[:, ncx, :],
                        start=(ncx == 0),
                        stop=(ncx == 1),
                    )
                nc.scalar.copy(g_sb[:, ic, :], g_ps[:])

            # ---- spill G (pool queue: input prefetch on sync is never blocked,
            # and the gathers that consume it are queued right behind)
            g_d = dram_pool.tile([2, 128, N], f32, tag="gd")
            nc.gpsimd.dma_start(g_d.rearrange("c p j -> p c j"), g_sb[:])

            # ---- gather path: one 256B token per edge
            tok = tok_pool.tile([128, QG, TOKB], f32, tag="tok")
            tab_ap = g_d.rearrange("c p (t u) -> (c p t) u", u=TOKB)
            CH = 1024
            for h in range(EG // CH):
                nc.gpsimd.dma_gather(
                    out_ap=tok[:, (CH // 128) * h : (CH // 128) * (h + 1), :],
                    in_ap=tab_ap,
                    idxs_ap=ti_sb[:, (CH // 16) * h : (CH // 16) * (h + 1)],
                    num_idxs=CH,
                    num_idxs_reg=CH,
                    elem_size=TOKB,
                    single_packet=False,
                    queue_num=h % 4,
                )

            # ---- W build (after the spill: G reaches the gathers early; the
            # PE then fills the rest of the batch slot with the W matmuls)
            w_sb = w_pool.tile([128, 2, N], bf16, tag="wsb")
            for nc2 in range(2):
                w_ps = psum_pool.tile([128, N], f32, tag="wps")
                for kc in range(KC):
                    nc.tensor.matmul(
                        w_ps[:],
                        lhsT=wi_sb[:, kc, nc2 * 128 : (nc2 + 1) * 128],
                        rhs=ww_sb[:, kc, :],
                        start=(kc == 0),
                        stop=(kc == KC - 1),
                    )
                nc.scalar.copy(w_sb[:, nc2, :], w_ps[:])

            # ---- DVE: fused multiply-reduce partials
            prod = tok_pool.tile([128, QG, TOKB], bf16, tag="prod")
            nc.vector.tensor_tensor(
                out=prod[:], in0=tok[:], in1=m_sb[:], op=mybir.AluOpType.mult
            )
            nc.vector.tensor_reduce(
                out=red_sb[:, b : b + 1],
                in_=prod[:].rearrange("p a b -> p (a b)"),
                axis=mybir.AxisListType.X,
                op=mybir.AluOpType.add,
            )
            wg = w_pool.tile([128, 2, N], bf16, tag="wg")
            nc.vector.tensor_tensor(
                out=wg[:], in0=w_sb[:], in1=g_sb[:], op=mybir.AluOpType.mult
            )
            nc.vector.tensor_reduce(
                out=red_sb[:, BPC + b : BPC + b + 1],
                in_=wg[:].rearrange("p a b -> p (a b)"),
                axis=mybir.AxisListType.X,
                op=mybir.AluOpType.add,
            )
            nc.vector.tensor_reduce(
                out=red_sb[:, 2 * BPC + b : 2 * BPC + b + 1],
                in_=ew_sb[:],
                axis=mybir.AxisListType.X,
                op=mybir.AluOpType.add,
            )

        # ---- cross-partition reduce of all partials in one matmul
        red_ps = psum_pool.tile([1, 3 * BPC], f32, tag="redps")
        nc.tensor.matmul(
            red_ps[:], lhsT=ones_sb[:], rhs=red_sb[:], start=True, stop=True
        )
        fin = const_pool.tile([1, 3 * BPC], f32)
        nc.vector.tensor_copy(fin[:], red_ps[:])

        # loss_b = (ga_b + wg_b) / max(sw_b, 1e-8); out = sum_b loss_b
        sl = const_pool.tile([1, BPC], f32)
        nc.vector.tensor_tensor(
            out=sl[:], in0=fin[:, :BPC], in1=fin[:, BPC : 2 * BPC],
            op=mybir.AluOpType.add,
        )
        sw_cl = const_pool.tile([1, BPC], f32)
        nc.vector.tensor_scalar_max(sw_cl[:], fin[:, 2 * BPC :], 1e-8)
        rsw = const_pool.tile([1, BPC], f32)
        nc.vector.reciprocal(rsw[:], sw_cl[:])
        lb = const_pool.tile([1, BPC], f32)
        nc.vector.tensor_tensor(
            out=lb[:], in0=sl[:], in1=rsw[:], op=mybir.AluOpType.mult
        )
        tot = const_pool.tile([1, 1], f32)
        nc.vector.tensor_reduce(
            out=tot[:], in_=lb[:], axis=mybir.AxisListType.X, op=mybir.AluOpType.add
        )
        nc.sync.dma_start(out[:], tot[:])

    if not nc.is_finalized():
        nc.finalize()
    return nc


_NC_CACHE = {}


def _get_nc():
    if "nc" not in _NC_CACHE:
        _NC_CACHE["nc"] = _build_bass()
    return _NC_CACHE["nc"]


def _prep_in_maps(P, d_error, edge_i, edge_j, edge_w):
    P = np.asarray(P, dtype=np.float32)
    d_error = np.asarray(d_error, dtype=np.float32)
    edge_i = np.asarray(edge_i, dtype=np.int32)
    edge_j = np.asarray(edge_j, dtype=np.int32)
    edge_w = np.asarray(edge_w, dtype=np.float32)

    # P^T per batch, laid out [128, 2, N]: pt[b, p, c, :] = P[b, :, c*128+p]
    PT = np.ascontiguousarray(np.transpose(P, (0, 2, 1)))
    PT = np.ascontiguousarray(
        PT.reshape(B, 2, 128, N).transpose(0, 2, 1, 3)
    ).astype(NP_BF16)
    D = np.ascontiguousarray(
        d_error.reshape(2, 128, N).transpose(1, 0, 2)
    ).astype(NP_BF16)

    # ---- gather path (first EG edges): token idx + mask
    fg = edge_i[:, :EG] * N + edge_j[:, :EG]  # [B, EG]
    tok_idx = (fg >> 6).astype(np.int16)
    # wrapped layout [B, 16, EG//16] (idx e' at [e'%16, e'//16]), tiled to 128
    ti = np.ascontiguousarray(
        tok_idx.reshape(B, EG // 16, 16).transpose(0, 2, 1)
    )
    ti = np.tile(ti, (1, 8, 1))  # [B, 128, EG//16]
    # mask M[b, e'%128, e'//128, lane] = w_e  (lane = fg & 127)
    lane = (fg & 63).astype(np.int64)
    wbf = edge_w[:, :EG].astype(NP_FP8)
    M = np.zeros((B, 128, QG, TOKB), dtype=NP_FP8)
    bidx = np.arange(B)[:, None]
    eidx = np.arange(EG)[None, :]
    M[bidx, eidx % 128, eidx // 128, lane] = wbf

    # ---- scatter-matmul path (remaining EW edges): fp8 one-hots
    i2 = edge_i[:, EG:].astype(np.int64)
    j2 = edge_j[:, EG:].astype(np.int64)
    w2 = edge_w[:, EG:]
    ONE_FP8 = np.float32(1.0).astype(NP_FP8)
    Wi = np.zeros((B, KC, 128, N), dtype=NP_FP8)
    Ww = np.zeros((B, KC, 128, N), dtype=NP_FP8)
    kidx = (np.arange(EW) // 128)[None, :]
    elidx = (np.arange(EW) % 128)[None, :]
    Wi[bidx, kidx, elidx, i2] = ONE_FP8
    Ww[bidx, kidx, elidx, j2] = w2.astype(NP_FP8)
    Wi = np.ascontiguousarray(Wi.transpose(0, 2, 1, 3))  # [B, 128, KC, N]
    Ww = np.ascontiguousarray(Ww.transpose(0, 2, 1, 3))

    # full edge weights for the denominator: edge e at [e%128, e//128]
    ew_l = np.ascontiguousarray(
        edge_w.reshape(B, E // 128, 128).transpose(0, 2, 1)
    )

    in_maps = []
    for c in range(NCORES):
        sl = slice(c * BPC, (c + 1) * BPC)
        in_maps.append(
            {
                "pt": np.ascontiguousarray(PT[sl]),
                "derr": D,
                "ti": np.ascontiguousarray(ti[sl]),
                "mk": np.ascontiguousarray(M[sl]),
                "wi": np.ascontiguousarray(Wi[sl]),
                "ww": np.ascontiguousarray(Ww[sl]),
                "ew": np.ascontiguousarray(ew_l[sl]),
            }
        )
    return in_maps


def run(P, d_error, edge_i, edge_j, edge_w, trace=False):
    """Run on 8 cores; returns (loss_scalar, BassKernelResults)."""
    nc = _get_nc()
    in_maps = _prep_in_maps(P, d_error, edge_i, edge_j, edge_w)
    res = run_bass_kernel_spmd(
        nc, in_maps, core_ids=list(range(NCORES)), trace=trace
    )
    partials = [r["out"].reshape(()) for r in res.results]
    loss = np.float32(np.sum(np.stack(partials), dtype=np.float64) / B)
    return loss, res


def kernel(P, d_error, edge_i, edge_j, edge_w):
    loss, _ = run(P, d_error, edge_i, edge_j, edge_w, trace=False)
    return np.asarray(loss, dtype=np.float32)
